# revision 14
# baseline (speedup 1.0000x reference)
"""Depthwise 3x3 + pointwise 1x1 conv (both 4-bit fake-quant weights) on 8 trn2 cores.

Data-parallel over batch (32 -> 8 cores x 4 images); per core,
channels-on-partitions (384 ch = 3 groups of 128). Host pre-pads x to 58x58
(zero border) and converts to fp16.

Design (evolved from the original baseline via TimelineSim + HW slope
measurements):
 - Bias folded upstream: c = pw_q^{-1} pw_b solved on host (fp64), added
   per-channel to the depthwise output y (free: activation-bias AP on the
   PE-path PSUM evacuation; dual-op tensor_scalar on the DVE-path first tap).
   The pointwise PSUM evacuation becomes a pure copy and
   z = pw_q @ (y + c) = pw_q y + b exactly.
 - z stored/DMA'd as fp16 (halves output traffic + evac cost); host converts
   to fp32. Adds ~1e-4 rel err against a 2e-2 budget.
 - Depthwise split three ways to balance engine busy-time (PE ~= DVE ~= the
   pole; ScalarE/DMA under): per (image, group), rows [0,HS) on TensorE
   (9 accumulating diag matmuls per 8-row PSUM chunk), rows [HS,56) on
   VectorE (tensor_scalar 4x products + tensor_tensor 2x adds), with the two
   odd-aligned center-column taps (0,1),(1,1) computed on ScalarE
   (activation Copy*scale at 1x, alignment-free) and added by DVE.
 - Tap (2,1) reads a second, element-shifted DRAM load of the band rows so
   its DVE product stays 4B-aligned (4x mode).
 - One unified y tile per (image, group): ScalarE evacuates PE rows, DVE
   writes band rows; pw chunks read one source, always exactly 3 matmuls.
 - Software-pipelined emission (dw(i+1) before pw(i)); pointwise emitted
   r0-major across the 3 output-channel groups (avoids PE head-of-line
   blocking on the last DVE band); z staged per (image, mg), DMA'd in 2
   halves.
 - Rejected by measurement: GPSIMD tensor ops (SBUF contention with DVE
   2-port modes), ldweights hoisting (already hidden), PE-heavier or
   DVE-heavier row splits.
"""

import numpy as np

# Problem shape (hardcoded per contract).
B_TOTAL, C, H, W = 32, 384, 56, 56
N_CORES = 8
B = B_TOTAL // N_CORES          # images per core
HP = H + 2                      # padded spatial
CG = C // 128                   # channel groups (contraction groups)
MG = C // 128                   # output-channel groups
P = 128

# Depthwise rows computed on TensorE per (image, channel-group); the rest of
# the 56 rows go to VectorE.
HS_PE = [
    [28, 28, 24],
    [28, 28, 24],
    [28, 28, 24],
    [36, 32, 32],
]
HB2_MAX = HP - min(min(r) for r in HS_PE)
HB_MAX = H - min(min(r) for r in HS_PE)
CHUNK_ROWS = 8                  # rows per PSUM chunk (fp32 bank: 512 elems)
CHUNK = CHUNK_ROWS * W          # 448 columns


def _row_chunks(total, step=CHUNK_ROWS):
    out, r = [], 0
    while r < total:
        n = min(step, total - r)
        out.append((r, n))
        r += n
    return out


TAPS = [(dh, dw) for dh in range(3) for dw in range(3)]

WEIGHT_BITS = 4
SCALE_MIN = np.float32(2e-16)


def _fake_quant(w: np.ndarray, bits: int = WEIGHT_BITS) -> np.ndarray:
    """Forward value of brevitas-style per-channel symmetric narrow int quant."""
    w = w.astype(np.float32)
    qmax = np.float32(2 ** (bits - 1) - 1)
    absmax = np.max(np.abs(w.reshape(w.shape[0], -1)), axis=1)
    scale = np.maximum(absmax / qmax, SCALE_MIN).astype(np.float32)
    scale = scale.reshape((-1,) + (1,) * (w.ndim - 1))
    q = np.clip(np.round(w / scale), -qmax, qmax).astype(np.float32) * scale
    return q.astype(np.float32)


def _build_nc(reps: int = 1, hw_loop: int = 0, no_io: bool = False):
    """no_io=True builds a DMA-free variant (no x loads, no z stores) for
    contention experiments -- compute reads whatever is resident in SBUF."""
    import concourse.bass as bass  # noqa: F401
    import concourse.tile as tile
    from concourse import bacc, mybir

    dt = mybir.dt
    f32, f16 = dt.float32, dt.float16
    Alu = mybir.AluOpType
    Act = mybir.ActivationFunctionType

    nc = bacc.Bacc("TRN2", target_bir_lowering=False, debug=False,
                   num_devices=N_CORES)

    # x arrives host-padded and host-converted: [B, C, 58, 58] fp16, zero
    # borders.
    x_d = nc.dram_tensor("x", [B, C, HP, HP], f16, kind="ExternalInput").ap()
    dwdiag_d = nc.dram_tensor("dwdiag", [P, CG * 9 * P], f16,
                              kind="ExternalInput").ap()
    pwT_d = nc.dram_tensor("pwT", [P, CG * MG * P], f16,
                           kind="ExternalInput").ap()
    taps_d = nc.dram_tensor("taps", [P, CG * 9], f32, kind="ExternalInput").ap()
    cvec_d = nc.dram_tensor("cvec", [P, CG], f32, kind="ExternalInput").ap()
    z_d = nc.dram_tensor("z", [B, C, H, W], f16, kind="ExternalOutput").ap()

    with tile.TileContext(nc) as tc:
        from contextlib import ExitStack, nullcontext
        with ExitStack() as ctx:
            consts = ctx.enter_context(tc.tile_pool(name="consts", bufs=1))
            xpad = ctx.enter_context(tc.tile_pool(name="xpad", bufs=8))
            xshp = ctx.enter_context(tc.tile_pool(name="xsh", bufs=6))
            yp = ctx.enter_context(tc.tile_pool(name="y", bufs=6))
            zstp = ctx.enter_context(tc.tile_pool(name="zst", bufs=6))
            tmpp = ctx.enter_context(tc.tile_pool(name="tmp", bufs=4))
            upool = ctx.enter_context(tc.tile_pool(name="u", bufs=2))
            dwps = ctx.enter_context(tc.tile_pool(name="dwps", bufs=4,
                                                  space="PSUM"))
            pwps = ctx.enter_context(tc.tile_pool(name="pwps", bufs=4,
                                                  space="PSUM"))

            dwdiag_t = consts.tile([P, CG * 9 * P], f16)
            nc.sync.dma_start(out=dwdiag_t[:], in_=dwdiag_d[:])
            taps_t = consts.tile([P, CG * 9], f32)
            nc.sync.dma_start(out=taps_t[:], in_=taps_d[:])
            cvec_t = consts.tile([P, CG], f32)
            nc.sync.dma_start(out=cvec_t[:], in_=cvec_d[:])
            # pwT load emitted after image 0/1 input loads (not needed until
            # the first pointwise) to unblock input DMA + PE sooner.
            pwT_t = consts.tile([P, CG * MG * P], f16)
            pw_consts_loaded = [False]

            def load_pw_consts():
                if not pw_consts_loaded[0]:
                    nc.sync.dma_start(out=pwT_t[:], in_=pwT_d[:])
                    pw_consts_loaded[0] = True

            loop_cm = (tc.For_i(0, hw_loop, 1,
                                hint_engines=(mybir.EngineType.PE,
                                              mybir.EngineType.DVE,
                                              mybir.EngineType.Activation,
                                              mybir.EngineType.Pool,
                                              mybir.EngineType.SP))
                       if hw_loop else nullcontext())
            with loop_cm:
              for rep in range(reps):
                xp_t = [[None] * CG for _ in range(B)]
                xs_t = [[None] * CG for _ in range(B)]
                y_t = [[None] * CG for _ in range(B)]

                def emit_loads(i):
                    for g in range(CG):
                        HS = HS_PE[i][g]
                        HB2 = HP - HS
                        xp = xpad.tile([P, HP * HP], f16)
                        xsrc = x_d[i, g * P:(g + 1) * P, :, :].rearrange(
                            "c a b -> c (a b)")
                        HSPLIT = 18 * HP
                        if not no_io:
                            nc.sync.dma_start(out=xp[:, :HSPLIT],
                                              in_=xsrc[:, :HSPLIT])
                            nc.sync.dma_start(out=xp[:, HSPLIT:],
                                              in_=xsrc[:, HSPLIT:])
                        else:
                            nc.vector.memset(xp[:, :2], 0)
                        # shifted-by-one-element copy of the band rows so the
                        # dw=1 taps read 4B-aligned (direct second DRAM load)
                        if i < B - 1:
                            xs = xshp.tile([P, HB2_MAX * HP], f16)
                            nsh = HB2 * HP - 1
                            if not no_io:
                                nc.sync.dma_start(
                                    out=xs[:, :nsh],
                                    in_=xsrc[:, HS * HP + 1:
                                             HS * HP + 1 + nsh])
                            else:
                                nc.vector.memset(xs[:, :2], 0)
                        else:
                            xs = None
                        xp_t[i][g], xs_t[i][g] = xp, xs

                def emit_dw(i):
                    for g in range(CG):
                        HS = HS_PE[i][g]
                        HB = H - HS
                        E = HB * W
                        xp, xs = xp_t[i][g], xs_t[i][g]
                        xp3 = xp[:, :].rearrange("p (a b) -> p a b", a=HP)
                        xs3 = (xs[:, :(HP - HS) * HP].rearrange(
                            "p (a b) -> p a b", a=HP - HS)
                               if xs is not None else None)
                        yt = yp.tile([P, H * W], f16)
                        y_t[i][g] = yt
                        cg_ap = cvec_t[:, g:g + 1]

                        # --- TensorE rows [0, HS): diag matmuls per tap ---
                        for h0, nr in _row_chunks(HS):
                            ps = dwps.tile([P, CHUNK], f32)
                            n = nr * W
                            for t, (dh, dw) in enumerate(TAPS):
                                rhs = xp3[:, h0 + dh:h0 + dh + nr, dw:dw + W]
                                lhsT = dwdiag_t[:, (g * 9 + t) * P:
                                                (g * 9 + t + 1) * P]
                                nc.tensor.matmul(ps[:, :n], lhsT=lhsT, rhs=rhs,
                                                 start=(t == 0),
                                                 stop=(t == len(TAPS) - 1))
                            # evacuation adds the folded bias c (per channel)
                            nc.scalar.activation(
                                out=yt[:, h0 * W:h0 * W + n],
                                in_=ps[:, :n],
                                func=Act.Identity,
                                bias=cg_ap,
                                scale=1.0,
                            )

                        # --- VectorE rows [HS, 56) ---
                        yb3 = yt[:, HS * W:HS * W + E].rearrange(
                            "p (a b) -> p a b", a=HB)
                        ybf = yt[:, HS * W:HS * W + E]

                        def band_ap(dh, dw):
                            if dw == 1:
                                return xs3[:, dh:dh + HB, 0:W]
                            return xp3[:, HS + dh:HS + dh + HB, dw:dw + W]

                        sc = lambda t: taps_t[:, g * 9 + t:g * 9 + t + 1]  # noqa: E731
                        # Odd-column tap products go to ScalarE (1x rate,
                        # no alignment constraint): taps (0,1),(1,1) always,
                        # plus (2,1) on the last image (so its shifted load is
                        # dropped). The last image's band runs as two
                        # half-chains so pw(3) chunks unblock earlier.
                        last = (i == B - 1)
                        if last:
                            mid = HS + ((H - HS) // 4) * 2
                            halves = [(HS, mid), (mid, H)]
                        else:
                            halves = [(HS, H)]
                        scal_taps = (1, 4, 7) if last else (1, 4)
                        for r_lo, r_hi in halves:
                            HBx = r_hi - r_lo
                            Ex = HBx * W
                            ybx = yt[:, r_lo * W:r_lo * W + Ex]
                            yb3x = ybx.rearrange("p (a b) -> p a b", a=HBx)

                            def bx(dh, dw, r_lo=r_lo, HBx=HBx):
                                if dw == 1:
                                    return xs3[:, r_lo - HS + dh:
                                               r_lo - HS + dh + HBx, 0:W]
                                return xp3[:, r_lo + dh:r_lo + dh + HBx,
                                           dw:dw + W]

                            us = []
                            for t in scal_taps:
                                dh = TAPS[t][0]
                                u = upool.tile([P, HB_MAX * W], f16,
                                               name=f"u{t}")
                                u3 = u[:, :Ex].rearrange("p (a b) -> p a b",
                                                         a=HBx)
                                nc.scalar.mul(
                                    u3[:, :, :],
                                    xp3[:, r_lo + dh:r_lo + dh + HBx,
                                        1:1 + W], sc(t))
                                us.append(u)
                            # first tap fused with the +c bias fold
                            nc.vector.tensor_scalar(
                                out=yb3x[:, :, :], in0=bx(0, 0),
                                scalar1=sc(0), scalar2=cg_ap,
                                op0=Alu.mult, op1=Alu.add)
                            for t, (dh, dw) in enumerate(TAPS):
                                if t == 0 or t in scal_taps:
                                    continue
                                tmp = tmpp.tile([P, HB_MAX * W], f16)
                                tmp3 = tmp[:, :Ex].rearrange(
                                    "p (a b) -> p a b", a=HBx)
                                nc.vector.tensor_scalar_mul(
                                    tmp3[:, :, :], bx(dh, dw), sc(t))
                                nc.vector.tensor_tensor(ybx, ybx, tmp[:, :Ex],
                                                        op=Alu.add)
                            for u in us:
                                nc.vector.tensor_tensor(ybx, ybx, u[:, :Ex],
                                                        op=Alu.add)

                def emit_pw(i):
                    load_pw_consts()
                    zts = []
                    for _mg in range(MG):
                        zt_mg = zstp.tile([P, H * W], f16, name="zt")
                        zts.append(zt_mg)
                    ZSPLIT = 32 * W
                    for r0, nr in _row_chunks(H):
                        n = nr * W
                        for mg in range(MG):
                            ps = pwps.tile([P, CHUNK], f32)
                            for kg in range(CG):
                                nc.tensor.matmul(
                                    ps[:, :n],
                                    lhsT=pwT_t[:, (kg * MG + mg) * P:
                                               (kg * MG + mg + 1) * P],
                                    rhs=y_t[i][kg][:, r0 * W:r0 * W + n],
                                    start=(kg == 0),
                                    stop=(kg == CG - 1),
                                )
                            nc.scalar.copy(out=zts[mg][:, r0 * W:r0 * W + n],
                                           in_=ps[:, :n])
                        if r0 + nr == 32 and not no_io:
                            for mg in range(MG):
                                nc.sync.dma_start(
                                    out=z_d[i, mg * P:(mg + 1) * P, :32, :]
                                    .rearrange("c a b -> c (a b)"),
                                    in_=zts[mg][:, :ZSPLIT],
                                )
                    if not no_io:
                        for mg in range(MG):
                            nc.sync.dma_start(
                                out=z_d[i, mg * P:(mg + 1) * P, 32:, :]
                                .rearrange("c a b -> c (a b)"),
                                in_=zts[mg][:, ZSPLIT:],
                            )

                emit_loads(0)
                emit_loads(1)
                emit_dw(0)
                emit_loads(2)
                emit_dw(1)
                emit_pw(0)
                emit_loads(3)
                emit_dw(2)
                emit_pw(1)
                emit_dw(3)
                emit_pw(2)
                emit_pw(3)

    nc.compile()
    return nc


def _host_consts(dw_w: np.ndarray, pw_w: np.ndarray, pw_b: np.ndarray):
    dw_q = _fake_quant(dw_w)                      # [384, 1, 3, 3]
    pw_q = _fake_quant(pw_w)                      # [384, 384, 1, 1]

    # taps [128, CG*9]: [c, g*9 + t] = dw_q[g*128 + c, 0, dh, dw]
    taps = (dw_q[:, 0].reshape(C, 9).reshape(CG, P, 9)
            .transpose(1, 0, 2).reshape(P, CG * 9).astype(np.float32))
    taps = np.ascontiguousarray(taps)

    # dwdiag [128, CG*9*128] fp16: block (g*9+t) = diag of that tap's weights
    eye = np.eye(P, dtype=np.float16)
    blocks = []
    for g in range(CG):
        for t in range(9):
            d = taps[:, g * 9 + t].astype(np.float16)
            blocks.append(eye * d[:, None])
    dwdiag = np.ascontiguousarray(np.concatenate(blocks, axis=1))

    # pwT [128, CG*MG*128] fp16: block (kg*MG+mg)[k, m] = pw_q[mg*128+m, kg*128+k]
    pw2 = pw_q[:, :, 0, 0]
    blocks = []
    for kg in range(CG):
        for mg in range(MG):
            blocks.append(np.ascontiguousarray(
                pw2[mg * P:(mg + 1) * P, kg * P:(kg + 1) * P].T.astype(np.float16)))
    pwT = np.ascontiguousarray(np.concatenate(blocks, axis=1))

    # folded bias: c solves pw_q @ c = b, so z = pw_q @ (y + c) = pw_q y + b.
    c = np.linalg.solve(pw2.astype(np.float64),
                        pw_b.astype(np.float64)).astype(np.float32)
    cvec = np.ascontiguousarray(c.reshape(CG, P).T.astype(np.float32))
    return dwdiag, pwT, taps, cvec


def _prepare_in_maps(x, dw_w, pw_w, pw_b):
    dwdiag, pwT, taps, cvec = _host_consts(dw_w, pw_w, pw_b)

    x = np.asarray(x, dtype=np.float32)
    xp = np.zeros((B_TOTAL, C, HP, HP), dtype=np.float16)
    xp[:, :, 1:H + 1, 1:W + 1] = x.astype(np.float16)
    shards = xp.reshape(N_CORES, B, C, HP, HP)
    return [
        {"x": np.ascontiguousarray(shards[c]), "dwdiag": dwdiag, "pwT": pwT,
         "taps": taps, "cvec": cvec}
        for c in range(N_CORES)
    ]


_NC_CACHE = None


def kernel(x: np.ndarray, dw_w: np.ndarray, pw_w: np.ndarray,
           pw_b: np.ndarray) -> np.ndarray:
    from concourse.bass_utils import run_bass_kernel_spmd

    global _NC_CACHE
    if _NC_CACHE is None:
        _NC_CACHE = _build_nc()
    nc = _NC_CACHE

    in_maps = _prepare_in_maps(x, dw_w, pw_w, pw_b)
    res = run_bass_kernel_spmd(nc, in_maps, list(range(N_CORES)))
    z = np.concatenate([res.results[c]["z"] for c in range(N_CORES)], axis=0)
    return z.astype(np.float32)


# revision 15
# speedup vs baseline: 1.0421x; 1.0421x over previous
"""Depthwise 3x3 + pointwise 1x1 conv (both 4-bit fake-quant weights) on 8 trn2 cores.

Data-parallel over batch (32 -> 8 cores x 4 images); per core,
channels-on-partitions (384 ch = 3 groups of 128). Host pre-pads x to 58x58
(zero border) and converts to fp16.

Design (evolved from the original baseline via TimelineSim + HW slope
measurements):
 - Bias folded upstream: c = pw_q^{-1} pw_b solved on host (fp64), added
   per-channel to the depthwise output y (free: activation-bias AP on the
   PE-path PSUM evacuation; dual-op tensor_scalar on the DVE-path first tap).
   The pointwise PSUM evacuation becomes a pure copy and
   z = pw_q @ (y + c) = pw_q y + b exactly.
 - z stored/DMA'd as fp16 (halves output traffic + evac cost); host converts
   to fp32. Adds ~1e-4 rel err against a 2e-2 budget.
 - Depthwise split three ways to balance engine busy-time (PE ~= DVE ~= the
   pole; ScalarE/DMA under): per (image, group), rows [0,HS) on TensorE
   (9 accumulating diag matmuls per 8-row PSUM chunk), rows [HS,56) on
   VectorE (tensor_scalar 4x products + tensor_tensor 2x adds), with the two
   odd-aligned center-column taps (0,1),(1,1) computed on ScalarE
   (activation Copy*scale at 1x, alignment-free) and added by DVE.
 - Tap (2,1) reads a second, element-shifted DRAM load of the band rows so
   its DVE product stays 4B-aligned (4x mode).
 - One unified y tile per (image, group): ScalarE evacuates PE rows, DVE
   writes band rows; pw chunks read one source, always exactly 3 matmuls.
 - Software-pipelined emission (dw(i+1) before pw(i)); pointwise emitted
   r0-major across the 3 output-channel groups (avoids PE head-of-line
   blocking on the last DVE band); z staged per (image, mg), DMA'd in 2
   halves.
 - Rejected by measurement: GPSIMD tensor ops (SBUF contention with DVE
   2-port modes), ldweights hoisting (already hidden), PE-heavier or
   DVE-heavier row splits.
"""

import numpy as np

# Problem shape (hardcoded per contract).
B_TOTAL, C, H, W = 32, 384, 56, 56
N_CORES = 8
B = B_TOTAL // N_CORES          # images per core
HP = H + 2                      # padded spatial
CG = C // 128                   # channel groups (contraction groups)
MG = C // 128                   # output-channel groups
P = 128

# Depthwise rows computed on TensorE per (image, channel-group); the rest of
# the 56 rows go to VectorE.
HS_PE = [
    [28, 28, 24],
    [28, 28, 24],
    [28, 28, 24],
    [36, 32, 32],
]
HB2_MAX = HP - min(min(r) for r in HS_PE)
HB_MAX = H - min(min(r) for r in HS_PE)
CHUNK_ROWS = 8                  # rows per PSUM chunk (fp32 bank: 512 elems)
CHUNK = CHUNK_ROWS * W          # 448 columns


def _row_chunks(total, step=CHUNK_ROWS):
    out, r = [], 0
    while r < total:
        n = min(step, total - r)
        out.append((r, n))
        r += n
    return out


TAPS = [(dh, dw) for dh in range(3) for dw in range(3)]

WEIGHT_BITS = 4
SCALE_MIN = np.float32(2e-16)


def _fake_quant(w: np.ndarray, bits: int = WEIGHT_BITS) -> np.ndarray:
    """Forward value of brevitas-style per-channel symmetric narrow int quant."""
    w = w.astype(np.float32)
    qmax = np.float32(2 ** (bits - 1) - 1)
    absmax = np.max(np.abs(w.reshape(w.shape[0], -1)), axis=1)
    scale = np.maximum(absmax / qmax, SCALE_MIN).astype(np.float32)
    scale = scale.reshape((-1,) + (1,) * (w.ndim - 1))
    q = np.clip(np.round(w / scale), -qmax, qmax).astype(np.float32) * scale
    return q.astype(np.float32)


def _build_nc(reps: int = 1, hw_loop: int = 0, no_io: bool = False):
    """no_io=True builds a DMA-free variant (no x loads, no z stores) for
    contention experiments -- compute reads whatever is resident in SBUF."""
    import concourse.bass as bass  # noqa: F401
    import concourse.tile as tile
    from concourse import bacc, mybir

    dt = mybir.dt
    f32, f16 = dt.float32, dt.float16
    Alu = mybir.AluOpType
    Act = mybir.ActivationFunctionType

    nc = bacc.Bacc("TRN2", target_bir_lowering=False, debug=False,
                   num_devices=N_CORES)

    # x arrives host-padded and host-converted: [B, C, 58, 58] fp16, zero
    # borders.
    x_d = nc.dram_tensor("x", [B, C, HP, HP], f16, kind="ExternalInput").ap()
    dwdiag_d = nc.dram_tensor("dwdiag", [P, CG * 9 * P], f16,
                              kind="ExternalInput").ap()
    pwT_d = nc.dram_tensor("pwT", [P, CG * MG * P], f16,
                           kind="ExternalInput").ap()
    taps_d = nc.dram_tensor("taps", [P, CG * 9], f32, kind="ExternalInput").ap()
    cvec_d = nc.dram_tensor("cvec", [P, CG], f32, kind="ExternalInput").ap()
    z_d = nc.dram_tensor("z", [B, C, H, W], f16, kind="ExternalOutput").ap()

    with tile.TileContext(nc) as tc:
        from contextlib import ExitStack, nullcontext
        with ExitStack() as ctx:
            consts = ctx.enter_context(tc.tile_pool(name="consts", bufs=1))
            xpad = ctx.enter_context(tc.tile_pool(name="xpad", bufs=8))
            xshp = ctx.enter_context(tc.tile_pool(name="xsh", bufs=6))
            yp = ctx.enter_context(tc.tile_pool(name="y", bufs=6))
            zstp = ctx.enter_context(tc.tile_pool(name="zst", bufs=6))
            tmpp = ctx.enter_context(tc.tile_pool(name="tmp", bufs=4))
            upool = ctx.enter_context(tc.tile_pool(name="u", bufs=2))
            dwps = ctx.enter_context(tc.tile_pool(name="dwps", bufs=4,
                                                  space="PSUM"))
            pwps = ctx.enter_context(tc.tile_pool(name="pwps", bufs=4,
                                                  space="PSUM"))

            dwdiag_t = consts.tile([P, CG * 9 * P], f16)
            nc.sync.dma_start(out=dwdiag_t[:], in_=dwdiag_d[:])
            taps_t = consts.tile([P, CG * 9], f32)
            nc.sync.dma_start(out=taps_t[:], in_=taps_d[:])
            cvec_t = consts.tile([P, CG], f32)
            nc.sync.dma_start(out=cvec_t[:], in_=cvec_d[:])
            # pwT load emitted after image 0/1 input loads (not needed until
            # the first pointwise) to unblock input DMA + PE sooner.
            pwT_t = consts.tile([P, CG * MG * P], f16)
            pw_consts_loaded = [False]

            def load_pw_consts():
                if not pw_consts_loaded[0]:
                    nc.sync.dma_start(out=pwT_t[:], in_=pwT_d[:])
                    pw_consts_loaded[0] = True

            loop_cm = (tc.For_i(0, hw_loop, 1,
                                hint_engines=(mybir.EngineType.PE,
                                              mybir.EngineType.DVE,
                                              mybir.EngineType.Activation,
                                              mybir.EngineType.Pool,
                                              mybir.EngineType.SP))
                       if hw_loop else nullcontext())
            with loop_cm:
              for rep in range(reps):
                xp_t = [[None] * CG for _ in range(B)]
                xs_t = [[None] * CG for _ in range(B)]
                y_t = [[None] * CG for _ in range(B)]

                def emit_loads(i):
                    for g in range(CG):
                        HS = HS_PE[i][g]
                        HB2 = HP - HS
                        xp = xpad.tile([P, HP * HP], f16)
                        xsrc = x_d[i, g * P:(g + 1) * P, :, :].rearrange(
                            "c a b -> c (a b)")
                        HSPLIT = 18 * HP
                        if not no_io:
                            nc.sync.dma_start(out=xp[:, :HSPLIT],
                                              in_=xsrc[:, :HSPLIT])
                            nc.sync.dma_start(out=xp[:, HSPLIT:],
                                              in_=xsrc[:, HSPLIT:])
                        else:
                            nc.vector.memset(xp[:, :2], 0)
                        # shifted-by-one-element copy of the band rows so the
                        # dw=1 taps read 4B-aligned (direct second DRAM load)
                        xs = xshp.tile([P, HB2_MAX * HP], f16)
                        nsh = HB2 * HP - 1
                        if not no_io:
                            nc.sync.dma_start(
                                out=xs[:, :nsh],
                                in_=xsrc[:, HS * HP + 1:HS * HP + 1 + nsh])
                        else:
                            nc.vector.memset(xs[:, :2], 0)
                        xp_t[i][g], xs_t[i][g] = xp, xs

                def emit_dw(i):
                    for g in range(CG):
                        HS = HS_PE[i][g]
                        HB = H - HS
                        E = HB * W
                        xp, xs = xp_t[i][g], xs_t[i][g]
                        xp3 = xp[:, :].rearrange("p (a b) -> p a b", a=HP)
                        xs3 = xs[:, :(HP - HS) * HP].rearrange(
                            "p (a b) -> p a b", a=HP - HS)
                        yt = yp.tile([P, H * W], f16)
                        y_t[i][g] = yt
                        cg_ap = cvec_t[:, g:g + 1]

                        # --- TensorE rows [0, HS): diag matmuls per tap ---
                        for h0, nr in _row_chunks(HS):
                            ps = dwps.tile([P, CHUNK], f32)
                            n = nr * W
                            for t, (dh, dw) in enumerate(TAPS):
                                rhs = xp3[:, h0 + dh:h0 + dh + nr, dw:dw + W]
                                lhsT = dwdiag_t[:, (g * 9 + t) * P:
                                                (g * 9 + t + 1) * P]
                                nc.tensor.matmul(ps[:, :n], lhsT=lhsT, rhs=rhs,
                                                 start=(t == 0),
                                                 stop=(t == len(TAPS) - 1))
                            # evacuation adds the folded bias c (per channel)
                            nc.scalar.activation(
                                out=yt[:, h0 * W:h0 * W + n],
                                in_=ps[:, :n],
                                func=Act.Identity,
                                bias=cg_ap,
                                scale=1.0,
                            )

                        # --- VectorE rows [HS, 56) ---
                        yb3 = yt[:, HS * W:HS * W + E].rearrange(
                            "p (a b) -> p a b", a=HB)
                        ybf = yt[:, HS * W:HS * W + E]

                        def band_ap(dh, dw):
                            if dw == 1:
                                return xs3[:, dh:dh + HB, 0:W]
                            return xp3[:, HS + dh:HS + dh + HB, dw:dw + W]

                        sc = lambda t: taps_t[:, g * 9 + t:g * 9 + t + 1]  # noqa: E731
                        # taps (0,1) and (1,1) products on ScalarE (1x rate,
                        # no alignment constraint -> reads xpad odd windows
                        # directly); DVE adds them at the end of its chain.
                        u01 = upool.tile([P, HB_MAX * W], f16, name="u01")
                        u01_3 = u01[:, :E].rearrange("p (a b) -> p a b", a=HB)
                        nc.scalar.mul(u01_3[:, :, :],
                                      xp3[:, HS:HS + HB, 1:1 + W], sc(1))
                        u11 = upool.tile([P, HB_MAX * W], f16, name="u11")
                        u11_3 = u11[:, :E].rearrange("p (a b) -> p a b", a=HB)
                        nc.scalar.mul(u11_3[:, :, :],
                                      xp3[:, HS + 1:HS + 1 + HB, 1:1 + W],
                                      sc(4))
                        # first tap fused with the +c bias fold (dual-op TS)
                        nc.vector.tensor_scalar(
                            out=yb3[:, :, :], in0=band_ap(0, 0),
                            scalar1=sc(0), scalar2=cg_ap,
                            op0=Alu.mult, op1=Alu.add)
                        for t, (dh, dw) in enumerate(TAPS):
                            if t in (0, 1, 4):
                                continue
                            tmp = tmpp.tile([P, HB_MAX * W], f16)
                            tmp3 = tmp[:, :E].rearrange("p (a b) -> p a b",
                                                        a=HB)
                            nc.vector.tensor_scalar_mul(tmp3[:, :, :],
                                                        band_ap(dh, dw), sc(t))
                            nc.vector.tensor_tensor(ybf, ybf, tmp[:, :E],
                                                    op=Alu.add)
                        nc.vector.tensor_tensor(ybf, ybf, u01[:, :E],
                                                op=Alu.add)
                        nc.vector.tensor_tensor(ybf, ybf, u11[:, :E],
                                                op=Alu.add)

                def emit_pw(i):
                    load_pw_consts()
                    zts = []
                    for _mg in range(MG):
                        zt_mg = zstp.tile([P, H * W], f16, name="zt")
                        zts.append(zt_mg)
                    ZSPLIT = 32 * W
                    for r0, nr in _row_chunks(H):
                        n = nr * W
                        for mg in range(MG):
                            ps = pwps.tile([P, CHUNK], f32)
                            for kg in range(CG):
                                nc.tensor.matmul(
                                    ps[:, :n],
                                    lhsT=pwT_t[:, (kg * MG + mg) * P:
                                               (kg * MG + mg + 1) * P],
                                    rhs=y_t[i][kg][:, r0 * W:r0 * W + n],
                                    start=(kg == 0),
                                    stop=(kg == CG - 1),
                                )
                            nc.scalar.copy(out=zts[mg][:, r0 * W:r0 * W + n],
                                           in_=ps[:, :n])
                        if r0 + nr == 32 and not no_io:
                            for mg in range(MG):
                                nc.sync.dma_start(
                                    out=z_d[i, mg * P:(mg + 1) * P, :32, :]
                                    .rearrange("c a b -> c (a b)"),
                                    in_=zts[mg][:, :ZSPLIT],
                                )
                    if not no_io:
                        for mg in range(MG):
                            nc.sync.dma_start(
                                out=z_d[i, mg * P:(mg + 1) * P, 32:, :]
                                .rearrange("c a b -> c (a b)"),
                                in_=zts[mg][:, ZSPLIT:],
                            )

                emit_loads(0)
                emit_loads(1)
                emit_dw(0)
                emit_loads(2)
                emit_dw(1)
                emit_pw(0)
                emit_loads(3)
                emit_dw(2)
                emit_pw(1)
                emit_dw(3)
                emit_pw(2)
                emit_pw(3)

    nc.compile()
    return nc


def _host_consts(dw_w: np.ndarray, pw_w: np.ndarray, pw_b: np.ndarray):
    dw_q = _fake_quant(dw_w)                      # [384, 1, 3, 3]
    pw_q = _fake_quant(pw_w)                      # [384, 384, 1, 1]

    # taps [128, CG*9]: [c, g*9 + t] = dw_q[g*128 + c, 0, dh, dw]
    taps = (dw_q[:, 0].reshape(C, 9).reshape(CG, P, 9)
            .transpose(1, 0, 2).reshape(P, CG * 9).astype(np.float32))
    taps = np.ascontiguousarray(taps)

    # dwdiag [128, CG*9*128] fp16: block (g*9+t) = diag of that tap's weights
    eye = np.eye(P, dtype=np.float16)
    blocks = []
    for g in range(CG):
        for t in range(9):
            d = taps[:, g * 9 + t].astype(np.float16)
            blocks.append(eye * d[:, None])
    dwdiag = np.ascontiguousarray(np.concatenate(blocks, axis=1))

    # pwT [128, CG*MG*128] fp16: block (kg*MG+mg)[k, m] = pw_q[mg*128+m, kg*128+k]
    pw2 = pw_q[:, :, 0, 0]
    blocks = []
    for kg in range(CG):
        for mg in range(MG):
            blocks.append(np.ascontiguousarray(
                pw2[mg * P:(mg + 1) * P, kg * P:(kg + 1) * P].T.astype(np.float16)))
    pwT = np.ascontiguousarray(np.concatenate(blocks, axis=1))

    # folded bias: c solves pw_q @ c = b, so z = pw_q @ (y + c) = pw_q y + b.
    c = np.linalg.solve(pw2.astype(np.float64),
                        pw_b.astype(np.float64)).astype(np.float32)
    cvec = np.ascontiguousarray(c.reshape(CG, P).T.astype(np.float32))
    return dwdiag, pwT, taps, cvec


def _prepare_in_maps(x, dw_w, pw_w, pw_b):
    dwdiag, pwT, taps, cvec = _host_consts(dw_w, pw_w, pw_b)

    x = np.asarray(x, dtype=np.float32)
    xp = np.zeros((B_TOTAL, C, HP, HP), dtype=np.float16)
    xp[:, :, 1:H + 1, 1:W + 1] = x.astype(np.float16)
    shards = xp.reshape(N_CORES, B, C, HP, HP)
    return [
        {"x": np.ascontiguousarray(shards[c]), "dwdiag": dwdiag, "pwT": pwT,
         "taps": taps, "cvec": cvec}
        for c in range(N_CORES)
    ]


_NC_CACHE = None


def kernel(x: np.ndarray, dw_w: np.ndarray, pw_w: np.ndarray,
           pw_b: np.ndarray) -> np.ndarray:
    from concourse.bass_utils import run_bass_kernel_spmd

    global _NC_CACHE
    if _NC_CACHE is None:
        _NC_CACHE = _build_nc()
    nc = _NC_CACHE

    in_maps = _prepare_in_maps(x, dw_w, pw_w, pw_b)
    res = run_bass_kernel_spmd(nc, in_maps, list(range(N_CORES)))
    z = np.concatenate([res.results[c]["z"] for c in range(N_CORES)], axis=0)
    return z.astype(np.float32)
